# revision 25
# baseline (speedup 1.0000x reference)
"""Self-attention block (B=16, S=1024, C=512, H=8, D=64) on 8 NeuronCores.

Data-parallel over batch: core i handles batches [2i, 2i+1]. No collectives.

Per-core pipeline (all on-chip after the initial DMAs):
  qkv proj -> q,k feature-major [d, s] bf16; v token-major fp8e4 scaled by
  16 with a 16.0 ones column per head (so P@V also yields 16x the softmax
  row-sums; the 16x cancels in the normalize). Scores are computed
  transposed S'[j, i] = k . q in bf16; P' = exp(scale * S') is written as
  fp8e4 (P in [e^-4, e^4], inside e4m3 normal range). P@V runs in fp8
  DoubleRow perf mode: each matmul contracts TWO 128-token chunks (3D APs
  [K, 2, dim], block layout) at 0.5 cycles/row -- half the PE time of the
  bf16 version. Deferred per-head normalization divides O^T rows by the
  row-sums; most heads bounce the sums through DRAM to respread them over
  128 lanes for the cheap DVE reciprocal; the last three heads (whose
  normalize gates the output projection) use an on-chip path instead:
  gpsimd partition_broadcast + DVE reciprocal_approx_fast (~18-bit), ~6us
  latency instead of ~13us. The value-path bias is folded through
  attention into the output bias (exact: softmax rows sum to 1).

Scheduling: fully software-pipelined. Each head's scores run TWO jc
chunks ahead of its P@V and the next head's first two scores chunks are
emitted right after the previous head's last P@V, so the ACT (exp) engine
always has a backlog and the PE never idles an exp latency at head
boundaries. All projection work (qkv chunks, v chunks, the previous
batch's output projection) is split into ~0.9us pieces interleaved into
the per-jc pipeline slots, sized to the ACT slack. The final batch's
output projection runs two-phase (cc=0..2 staged into SBUF inside head
7's slots, cc=3 added after), with the final stores split across both DMA
queues to halve the drain.

NOTE: the chip enforces a package power cap -- schedules that pack the PE
much past ~80% active trip a 50% utilization clamp and run slower.
"""

import numpy as np

import concourse.bacc as bacc
import concourse.tile as tile
import concourse.mybir as mybir
from concourse.bass_utils import run_bass_kernel_spmd

B, S, C, H, D = 16, 1024, 512, 8, 64
NCORES = 8
BPC = B // NCORES  # batches per core
F32 = mybir.dt.float32
ADT = mybir.dt.bfloat16

SCJ = 8  # S/128 chunks (token/key chunks)
CCH = 4  # C/128 chunks (model-dim chunks)
FCH = 8  # (2C)/128 chunks of q|k features
VW = H * (D + 1)  # 520: v row width per jc incl. ones column per head


def _register_ntff_hook():
    import sys, types

    if "antenv.axon_hooks" in sys.modules:
        return
    try:
        import trn_agent_boot.trn_boot as tb

        hook = [None]
        mod = types.ModuleType("antenv.axon_hooks")
        mod.set_axon_ntff_profile_hook = lambda h: hook.__setitem__(0, h)
        mod.get_axon_ntff_profile_hook = lambda: hook[0]
        sys.modules["antenv.axon_hooks"] = mod
        mod.set_axon_ntff_profile_hook(
            tb._ntff_profile_via_ctypes("/opt/axon/libaxon_pjrt.so")
        )
    except Exception:
        pass


def build():
    nc = bacc.Bacc("TRN2", target_bir_lowering=False, debug=False)

    xT = nc.declare_dram_parameter("xT", [BPC, C, S], ADT, isOutput=False)
    wqkvT = nc.declare_dram_parameter("wqkvT", [C, 3 * C], ADT, isOutput=False)
    wouT = nc.declare_dram_parameter("wouT", [C, C], ADT, isOutput=False)
    bqk = nc.declare_dram_parameter("bqk", [128, FCH], F32, isOutput=False)
    beff = nc.declare_dram_parameter("beff", [C], F32, isOutput=False)
    y = nc.declare_dram_parameter("y", [BPC, S, C], F32, isOutput=True)

    from contextlib import ExitStack

    with tile.TileContext(nc) as tc, ExitStack() as ctx:
        ctx.enter_context(
            nc.allow_low_precision(reason="bf16/fp8 matmul operand staging")
        )
        consts = ctx.enter_context(tc.tile_pool(name="consts", bufs=1))
        xpool = ctx.enter_context(tc.tile_pool(name="x", bufs=2))
        qkpool = ctx.enter_context(tc.tile_pool(name="qkt", bufs=17))
        vpool = ctx.enter_context(tc.tile_pool(name="v", bufs=2))
        ppool = ctx.enter_context(tc.tile_pool(name="p", bufs=4))
        opool = ctx.enter_context(tc.tile_pool(name="o", bufs=2))
        rpool = ctx.enter_context(tc.tile_pool(name="r", bufs=4))
        spool = ctx.enter_context(tc.tile_pool(name="s", bufs=3))
        ypool = ctx.enter_context(tc.tile_pool(name="y", bufs=11))
        bcpool = ctx.enter_context(tc.tile_pool(name="bc", bufs=4))
        drpool = ctx.enter_context(tc.tile_pool(name="dr", bufs=4, space="DRAM"))
        ps_a = ctx.enter_context(tc.tile_pool(name="ps_a", bufs=3, space="PSUM"))
        ps_o = ctx.enter_context(tc.tile_pool(name="ps_o", bufs=1, space="PSUM"))

        # --- boot DMAs: the critical set (x, q third, k third) is split
        # across both queues so the first scores' deps land earliest.
        wq_sb = consts.tile([128, CCH * 3 * C], ADT)  # [c%128, cc*1536 + f]
        bqk_sb = consts.tile([128, FCH], F32)
        nc.sync.dma_start(out=bqk_sb, in_=bqk[:, :])
        x_tiles = [None, None]
        x_tiles[0] = xpool.tile([128, CCH * S], ADT, tag="x", name="x0")

        def _dma_x_chunk(eng, b, cc):
            eng.dma_start(
                out=x_tiles[b][:, cc * S : (cc + 1) * S],
                in_=xT[b][cc * 128 : (cc + 1) * 128, :],
            )

        def _dma_w_third(eng, cc, part):
            eng.dma_start(
                out=wq_sb[:, cc * 1536 + part * 512 : cc * 1536 + part * 512 + 512],
                in_=wqkvT[cc * 128 : (cc + 1) * 128, part * 512 : part * 512 + 512],
            )

        for cc in range(2):  # sync: x0, k0, q0, x1, k1, q1
            _dma_x_chunk(nc.sync, 0, cc)
            _dma_w_third(nc.sync, cc, 1)
            _dma_w_third(nc.sync, cc, 0)
        for cc in range(2, 4):  # gpsimd: x2, k2, q2, x3, k3, q3
            _dma_x_chunk(nc.gpsimd, 0, cc)
            _dma_w_third(nc.gpsimd, cc, 1)
            _dma_w_third(nc.gpsimd, cc, 0)
        for cc in range(CCH):  # v third
            _dma_w_third(nc.gpsimd, cc, 2)
        beff_sb = consts.tile([128, C], F32)
        nc.gpsimd.dma_start(out=beff_sb, in_=beff[:].partition_broadcast(128))
        wo_sb = consts.tile([128, CCH * C], ADT)  # [c%128, cc*512 + f]
        nc.sync.dma_start(
            out=wo_sb.rearrange("p (cc f) -> p cc f", cc=CCH),
            in_=wouT[:, :].rearrange("(cc p) f -> p cc f", p=128),
        )
        # f32 1.0 for the transpose-matmul identity in the fast normalize
        one_sb = consts.tile([1, 1], F32)
        nc.vector.memset(one_sb, 1.0)

        def emit_x(b):
            x_sb = xpool.tile([128, CCH * S], ADT, tag="x", name=f"x{b}")
            x_tiles[b] = x_sb
            for cc in range(CCH):
                _dma_x_chunk(nc.sync if cc % 2 == 0 else nc.gpsimd, b, cc)

        qk_tiles = {0: [None] * FCH, 1: [None] * FCH}
        qk_ps = {}

        def emit_qk_piece(b, fc, ih):
            # half of a q/k projection chunk; ih==1 also evacuates
            x_sb = x_tiles[b]
            if ih == 0:
                qk_ps[(b, fc)] = ps_a.tile(
                    [128, 1024], F32, tag="ps_a", name=f"psq{b}_{fc}"
                )
            ps = qk_ps[(b, fc)]
            for cc in range(CCH):
                nc.tensor.matmul(
                    ps[:, ih * 512 : (ih + 1) * 512],
                    lhsT=wq_sb[:, cc * 1536 + fc * 128 : cc * 1536 + (fc + 1) * 128],
                    rhs=x_sb[:, cc * S + ih * 512 : cc * S + ih * 512 + 512],
                    start=(cc == 0),
                    stop=(cc == CCH - 1),
                )
            if ih == 1:
                del qk_ps[(b, fc)]
                qt = qkpool.tile([128, S], ADT, tag="qkt", name=f"qkt{b}_{fc}")
                nc.vector.tensor_scalar_add(
                    out=qt, in0=ps[:, :], scalar1=bqk_sb[:, fc : fc + 1]
                )
                qk_tiles[b][fc] = qt

        def emit_qk_chunk(b, fc):
            emit_qk_piece(b, fc, 0)
            emit_qk_piece(b, fc, 1)

        v_tiles = [None, None]

        def emit_v_alloc(b):
            # v token-major bf16 [s%128, jc*520 + h*65 + d] with a ones
            # column per head (so P@V also yields the softmax row-sums)
            v_sb = vpool.tile([128, SCJ * VW], ADT, tag="v", name=f"v{b}")
            v_view = v_sb.rearrange("p (jc h dd) -> p jc h dd", jc=SCJ, h=H)
            # DVE, not gpsimd: the Q7 memset mishandles this strided AP on HW
            nc.vector.memset(v_view[:, :, :, D : D + 1], 1.0)
            v_tiles[b] = v_sb

        def emit_v_chunk(b, jc):
            x_sb = x_tiles[b]
            v_view = v_tiles[b].rearrange("p (jc h dd) -> p jc h dd", jc=SCJ, h=H)
            ps = ps_a.tile([128, 1024], F32, tag="ps_a", name=f"psv{b}_{jc}")
            for cc in range(CCH):
                nc.tensor.matmul(
                    ps[:, 0:512],
                    lhsT=x_sb[:, cc * S + jc * 128 : cc * S + (jc + 1) * 128],
                    rhs=wq_sb[:, cc * 1536 + 1024 : cc * 1536 + 1536],
                    start=(cc == 0),
                    stop=(cc == CCH - 1),
                )
            nc.vector.tensor_copy(
                out=v_view[:, jc, :, 0:D],
                in_=ps[:, 0:512].rearrange("p (h d) -> p h d", h=H),
            )

        pts = {}
        pos = {}
        o_sbs = {}
        sums_sbs = {}

        def emit_s(b, h, jc):
            # scores S'[j, i] = k . q (bf16), then P' = exp(scale*S')
            fq = h // 2
            fk = 4 + h // 2
            pb = (h % 2) * 64
            ps = ps_a.tile([128, 1024], F32, tag="ps_a", name=f"pss{b}_{h}_{jc}")
            for ih in range(2):
                nc.tensor.matmul(
                    ps[:, ih * 512 : (ih + 1) * 512],
                    lhsT=qk_tiles[b][fk][pb : pb + 64, jc * 128 : (jc + 1) * 128],
                    rhs=qk_tiles[b][fq][pb : pb + 64, ih * 512 : ih * 512 + 512],
                    start=True,
                    stop=True,
                )
            pt = ppool.tile([128, 1024], ADT, tag="p", name=f"pt{b}_{h}_{jc}")
            nc.scalar.activation(
                out=pt, in_=ps[:, :],
                func=mybir.ActivationFunctionType.Exp,
                scale=float(D) ** -0.5,
            )
            pts[(b, h, jc)] = pt

        def emit_p(b, h, jc):
            # O^T[d, i] += V_ext^T @ P'  (row 64 = row-sums)
            if jc == 0:
                pos[(b, h)] = ps_o.tile([65, 1024], F32, tag="ps_o", name=f"po{b}_{h}")
            po = pos[(b, h)]
            pt = pts.pop((b, h, jc))
            v_sb = v_tiles[b]
            for ih in range(2):
                nc.tensor.matmul(
                    po[:, ih * 512 : (ih + 1) * 512],
                    lhsT=v_sb[:, jc * VW + h * (D + 1) : jc * VW + (h + 1) * (D + 1)],
                    rhs=pt[:, ih * 512 : (ih + 1) * 512],
                    start=(jc == 0),
                    stop=(jc == SCJ - 1),
                )

        def emit_evac(b, h):
            # 16x row-sums first (they gate the normalize chain), then the
            # unnormalized O^T rows
            po = pos.pop((b, h))
            hh = h % 2
            sums_sb = spool.tile([1, S], F32, tag="sums", name=f"sm{b}_{h}")
            nc.vector.tensor_copy(out=sums_sb, in_=po[64:65, :])
            sums_sbs[(b, h)] = sums_sb
            o_sb = o_sbs[b]
            nc.vector.tensor_copy(
                out=o_sb[hh * 64 : (hh + 1) * 64, (h // 2) * S : (h // 2 + 1) * S],
                in_=po[0:64, :],
            )

        def emit_norm_bounce(b, h):
            # respread the row-sums over 128 lanes via a DRAM bounce (the
            # exact DVE reciprocal is ~8 cyc/elem/lane), broadcast back over
            # 64 partitions, multiply in place. ~13us latency, engine-cheap;
            # used for heads whose normalize has plenty of slack.
            hh = h % 2
            hp = h // 2
            sums_sb = sums_sbs.pop((b, h))
            sums_dr = drpool.tile([S], F32, tag="sdr", name=f"sdr{b}_{h}")
            nc.sync.dma_start(out=sums_dr[:].unsqueeze(0), in_=sums_sb)
            sums_sq = rpool.tile([128, S // 128], F32, tag="ssq", name=f"ssq{b}_{h}")
            nc.sync.dma_start(
                out=sums_sq, in_=sums_dr.rearrange("(p c) -> p c", p=128)
            )
            recs_sq = rpool.tile([128, S // 128], F32, tag="rsq", name=f"rsq{b}_{h}")
            nc.vector.reciprocal(out=recs_sq, in_=sums_sq)
            recs_dr = drpool.tile([S], F32, tag="rdr", name=f"rdr{b}_{h}")
            nc.sync.dma_start(
                out=recs_dr.rearrange("(p c) -> p c", p=128), in_=recs_sq
            )
            bc = bcpool.tile([128, S], F32, tag="bc", name=f"bc{b}_{h}")
            nc.sync.dma_start(
                out=bc[hh * 64 : (hh + 1) * 64, :],
                in_=recs_dr[:].partition_broadcast(64),
            )
            nc.vector.tensor_mul(
                out=o_sbs[b][hh * 64 : (hh + 1) * 64, hp * S : (hp + 1) * S],
                in0=o_sbs[b][hh * 64 : (hh + 1) * 64, hp * S : (hp + 1) * S],
                in1=bc[hh * 64 : (hh + 1) * 64, :],
            )

        def emit_norm_fast(b, h):
            # lower-latency normalize for the late heads that gate the output
            # projection: respread the sums row over 128 lanes with eight
            # tiny PE transpose-matmuls (saves the two front DMA hops of the
            # bounce, ~4us), then reciprocal + DRAM broadcast + multiply.
            hh = h % 2
            hp = h // 2
            sums_sb = sums_sbs.pop((b, h))
            # stride-8 transpose blocks so the respread lands row-major
            # ([p, c] = sums[p*8+c]): the DMA back out then writes 32-byte
            # runs per partition instead of 1024 scattered 4-byte elements
            sums_v = sums_sb.rearrange("o (p c) -> o p c", c=S // 128)
            ps1 = ps_a.tile([128, S // 128], F32, tag="ps_a", name=f"pst{b}_{h}")
            for c in range(S // 128):
                nc.tensor.matmul(
                    ps1[:, c : c + 1],
                    lhsT=sums_v[0:1, :, c],
                    rhs=one_sb[0:1, 0:1],
                    is_transpose=True,
                    start=True,
                    stop=True,
                )
            recs_sq = rpool.tile([128, S // 128], F32, tag="rsq", name=f"rsq{b}_{h}")
            nc.vector.reciprocal(out=recs_sq, in_=ps1)
            recs_dr = drpool.tile([S], F32, tag="rdr", name=f"rdr{b}_{h}")
            nc.sync.dma_start(
                out=recs_dr.rearrange("(p c) -> p c", p=128), in_=recs_sq
            )
            bc = bcpool.tile([128, S], F32, tag="bc", name=f"bc{b}_{h}")
            nc.sync.dma_start(
                out=bc[hh * 64 : (hh + 1) * 64, :],
                in_=recs_dr[:].partition_broadcast(64),
            )
            nc.vector.tensor_mul(
                out=o_sbs[b][hh * 64 : (hh + 1) * 64, hp * S : (hp + 1) * S],
                in0=o_sbs[b][hh * 64 : (hh + 1) * 64, hp * S : (hp + 1) * S],
                in1=bc[hh * 64 : (hh + 1) * 64, :],
            )

        FAST_NORM = {(1, 5), (1, 6), (1, 7)}

        def emit_norm(b, h):
            if (b, h) in FAST_NORM:
                emit_norm_fast(b, h)
            else:
                emit_norm_bounce(b, h)

        def emit_prologue(b, h):
            emit_s(b, h, 0)
            emit_s(b, h, 1)

        def emit_body(b, h, v_interleave=False, extras=None):
            for jc in range(SCJ):
                if v_interleave and jc < SCJ - 2:
                    emit_v_chunk(b, jc + 2)
                emit_p(b, h, jc)
                if jc + 2 < SCJ:
                    emit_s(b, h, jc + 2)
                if extras is not None and jc in extras:
                    for u in extras[jc]:
                        u()
            emit_evac(b, h)

        ob_ps = {}

        def emit_ob_piece(b, sc, part):
            # half of a full out-projection chunk for a finished batch
            o_sb = o_sbs[b]
            if part == 0:
                ob_ps[(b, sc)] = ps_a.tile(
                    [128, 512], F32, tag="ps_a", name=f"psy{b}_{sc}"
                )
            ps = ob_ps[(b, sc)]
            for cc in (0, 1) if part == 0 else (2, 3):
                nc.tensor.matmul(
                    ps[:, 0:512],
                    lhsT=o_sb[:, cc * S + sc * 128 : cc * S + (sc + 1) * 128],
                    rhs=wo_sb[:, cc * C : (cc + 1) * C],
                    start=(cc == 0),
                    stop=(cc == CCH - 1),
                )
            if part == 1:
                del ob_ps[(b, sc)]
                y_sb = ypool.tile([128, C], F32, tag="y", name=f"y{b}_{sc}")
                nc.vector.tensor_add(out=y_sb, in0=ps[:, 0:512], in1=beff_sb)
                # gpsimd queue only: keep sync free for the normalize hops
                nc.gpsimd.dma_start(out=y[b][sc * 128 : (sc + 1) * 128, :], in_=y_sb)

        ys = [None] * SCJ

        def emit_pyA(sc):
            # final batch out-proj, phase A: cc=0..2 staged into SBUF
            o_sb = o_sbs[BPC - 1]
            ps = ps_a.tile([128, 512], F32, tag="ps_a", name=f"pyA{sc}")
            for cc in range(CCH - 1):
                nc.tensor.matmul(
                    ps[:, 0:512],
                    lhsT=o_sb[:, cc * S + sc * 128 : cc * S + (sc + 1) * 128],
                    rhs=wo_sb[:, cc * C : (cc + 1) * C],
                    start=(cc == 0),
                    stop=(cc == CCH - 2),
                )
            y_sb = ypool.tile([128, C], F32, tag="y", name=f"yA{sc}")
            nc.vector.tensor_add(out=y_sb, in0=ps[:, 0:512], in1=beff_sb)
            ys[sc] = y_sb

        def emit_pyB(sc):
            # final batch out-proj, phase B: cc=3 added into the staged
            # tiles; stores split across both queues to halve the drain
            o_sb = o_sbs[BPC - 1]
            cc = CCH - 1
            ps = ps_a.tile([128, 512], F32, tag="ps_a", name=f"pyB{sc}")
            nc.tensor.matmul(
                ps[:, 0:512],
                lhsT=o_sb[:, cc * S + sc * 128 : cc * S + (sc + 1) * 128],
                rhs=wo_sb[:, cc * C : (cc + 1) * C],
                start=True,
                stop=True,
            )
            nc.vector.tensor_add(out=ys[sc], in0=ys[sc], in1=ps[:, 0:512])
            nc.gpsimd.dma_start(
                out=y[BPC - 1][sc * 128 : (sc + 1) * 128, 0:256], in_=ys[sc][:, 0:256]
            )
            nc.sync.dma_start(
                out=y[BPC - 1][sc * 128 : (sc + 1) * 128, 256:512], in_=ys[sc][:, 256:512]
            )

        # ---- main schedule -------------------------------------------------
        def qk_ab(b, fc):
            return (
                lambda: emit_qk_piece(b, fc, 0),
                lambda: emit_qk_piece(b, fc, 1),
            )

        def ob_ab(b, sc):
            return (
                lambda: emit_ob_piece(b, sc, 0),
                lambda: emit_ob_piece(b, sc, 1),
            )

        def four(p1, p2):
            # two 2-piece units spread over the head's jc slots
            return {1: [p1[0]], 3: [p1[1]], 5: [p2[0]], 7: [p2[1]]}

        def two(p1):
            return {3: [p1[0]], 7: [p1[1]]}

        extras_map = {
            (0, 1): four(qk_ab(0, 1), qk_ab(0, 5)),
            (0, 2): four(qk_ab(0, 2), qk_ab(0, 6)),
            (0, 3): four(qk_ab(0, 3), qk_ab(0, 7)),
            (0, 4): two(qk_ab(1, 0)),
            (0, 5): two(qk_ab(1, 4)),
            (0, 6): two(qk_ab(1, 1)),
            (0, 7): two(qk_ab(1, 5)),
            (1, 1): four(qk_ab(1, 2), qk_ab(1, 6)),
            (1, 2): four(qk_ab(1, 3), qk_ab(1, 7)),
            (1, 3): four(ob_ab(0, 0), ob_ab(0, 1)),
            (1, 4): four(ob_ab(0, 2), ob_ab(0, 3)),
            (1, 5): four(ob_ab(0, 4), ob_ab(0, 5)),
            (1, 6): four(ob_ab(0, 6), ob_ab(0, 7)),
            (1, 7): {jc: [lambda sc=jc: emit_pyA(sc)] for jc in range(SCJ)},
        }

        # boot: first k/q chunks (fc4 then fc0; the k third is the long
        # DMA pole so its matmuls lead). NOTE: PE warm-up junk matmuls were
        # tried here and are a big net loss -- the package power cap
        # charges every array op, and the extra work tripled the 50%
        # utilization clamp time.
        for fc, nj in ((4, 0), (0, 0)):
            ps = ps_a.tile([128, 1024], F32, tag="ps_a", name=f"psq0_{fc}")
            for cc in range(CCH):
                nc.tensor.matmul(
                    ps[:, 0:512],
                    lhsT=wq_sb[:, cc * 1536 + fc * 128 : cc * 1536 + (fc + 1) * 128],
                    rhs=x_tiles[0][:, cc * S : cc * S + 512],
                    start=(cc == 0),
                    stop=(cc == CCH - 1),
                )
            for cc in range(CCH):
                nc.tensor.matmul(
                    ps[:, 512:1024],
                    lhsT=wq_sb[:, cc * 1536 + fc * 128 : cc * 1536 + (fc + 1) * 128],
                    rhs=x_tiles[0][:, cc * S + 512 : cc * S + 1024],
                    start=(cc == 0),
                    stop=(cc == CCH - 1),
                )
            qt = qkpool.tile([128, S], ADT, tag="qkt", name=f"qkt0_{fc}")
            nc.vector.tensor_scalar_add(
                out=qt, in0=ps[:, :], scalar1=bqk_sb[:, fc : fc + 1]
            )
            qk_tiles[0][fc] = qt
        for b in range(BPC):
            o_sbs[b] = opool.tile([128, CCH * S], ADT, tag="o", name=f"o{b}")
            last_b = b == BPC - 1
            if b == 0:
                emit_v_alloc(0)
                emit_prologue(0, 0)
            for h in range(H):
                if h == 0:
                    emit_v_chunk(b, 0)
                    emit_v_chunk(b, 1)
                emit_body(
                    b, h,
                    v_interleave=(h == 0),
                    extras=extras_map.get((b, h)),
                )
                if b == 0 and h == 2:
                    emit_x(1)  # next batch's x DMA, early
                # prologue first so the fast-norm's PE transposes (which
                # wait on the DVE sums copy) queue behind the next head's
                # scores instead of stalling the PE
                if h < H - 1:
                    emit_prologue(b, h + 1)
                elif not last_b:
                    emit_v_alloc(b + 1)
                    emit_prologue(b + 1, 0)
                emit_norm(b, h)
        # tail: cc=3 contributions land in the staged phase-A tiles
        for sc in range(SCJ):
            emit_pyB(sc)

    nc.compile()
    return nc


_NC_CACHE = None
LAST_RESULT = None


def kernel(vis_feat, text_feat, w_qkv, b_qkv, w_out, b_out):
    global _NC_CACHE, LAST_RESULT
    _register_ntff_hook()
    if _NC_CACHE is None:
        _NC_CACHE = build()
    nc = _NC_CACHE

    adt_np = np.dtype(mybir.dt.np(ADT))
    vis_feat = np.asarray(vis_feat, dtype=np.float32)
    w_qkv = np.asarray(w_qkv, dtype=np.float32)
    b_qkv = np.asarray(b_qkv, dtype=np.float32)
    w_out = np.asarray(w_out, dtype=np.float32)
    b_out = np.asarray(b_out, dtype=np.float32)

    wqkvT = np.ascontiguousarray(w_qkv.T).astype(adt_np)  # [C, 3C]
    wouT = np.ascontiguousarray(w_out.T).astype(adt_np)  # [C, C]
    bqk = np.ascontiguousarray(b_qkv[: 2 * C].reshape(FCH, 128).T)  # [128, 8]
    beff = np.ascontiguousarray(b_out + b_qkv[2 * C :] @ w_out.T)  # [C]

    in_maps = []
    for i in range(NCORES):
        xTi = np.ascontiguousarray(
            vis_feat[i * BPC : (i + 1) * BPC].transpose(0, 2, 1)
        ).astype(adt_np)  # [BPC, C, S]
        in_maps.append(
            {"xT": xTi, "wqkvT": wqkvT, "wouT": wouT, "bqk": bqk, "beff": beff}
        )

    res = run_bass_kernel_spmd(nc, in_maps, core_ids=list(range(NCORES)))
    LAST_RESULT = res
    return np.concatenate([res.results[i]["y"] for i in range(NCORES)], axis=0)


# revision 29
# speedup vs baseline: 1.1647x; 1.1647x over previous
"""Self-attention block (B=16, S=1024, C=512, H=8, D=64) on 8 NeuronCores.

Data-parallel over batch: core i handles batches [2i, 2i+1]. No collectives.

Per-core pipeline (all on-chip after the initial DMAs):
  qkv proj -> q,k feature-major [d, s], v token-major [s, d] with a ones
  column per head (so P@V_ext also yields the softmax row-sums); scores
  computed transposed S'[j, i] = k . q so exp(S') feeds the P@V matmul
  directly as lhsT; softmax skips max-subtraction (logits bounded ~+-4);
  deferred per-head normalization divides O^T rows by the row-sums
  (respread over 128 lanes via a DRAM bounce for the DVE reciprocal,
  broadcast back, in-place multiply); output projection consumes the
  normalized heads straight out of SBUF. The value-path bias is folded
  through attention into the output bias (exact: softmax rows sum to 1).

Scheduling: initial DMAs interleave x chunks with q AND k thirds of
w_qkv per-cc across two queues (k was previously last and gated the
first scores by ~3us); the first two scores chunks of head 0 are emitted
right after the fc0/fc4 projections so the exp pipeline starts during
the remaining projections; batch 1's projection chunks fill the PE slack
between batch 0's attention pairs, batch 0's output projection fills the
same slots during batch 1; heads normalize right after their P@V
evacuation so the chain hides under the next head. Final-batch tail:
head 6's normalize bounces through DRAM while phase A of the output
projection (cc=0..2, staged into SBUF) runs between heads 6 and 7;
heads 5 and 7 use a lower-latency normalize whose lane-respread is eight
tiny PE transpose-matmuls instead of the two front DMA hops; after head
7, the last two batch-0 projection chunks cover the normalize chain, and
the cc=3 contributions land into the staged tiles with stores split
across both DMA queues.

Dtypes: bf16 matmuls throughout. fp8 (DoubleRow) was evaluated end to
end: softmax averaging shrinks signal as fast as quantization noise, so
the ~4% fp8e4m3 element error survives to the output (rel-err ~2e-2 vs
the 2e-2 gate) -- fundamentally unusable here. NOTE: the chip enforces a
package power cap with DVFS: schedules that pack the PE past ~80% busy
(removing the per-head pipeline-refill gaps, adding warm-up matmuls,
etc.) LOWER the average PE clock and run strictly slower -- the
per-head gaps are load-bearing; leave them.
"""

import numpy as np

import concourse.bacc as bacc
import concourse.tile as tile
import concourse.mybir as mybir
from concourse.bass_utils import run_bass_kernel_spmd

B, S, C, H, D = 16, 1024, 512, 8, 64
NCORES = 8
BPC = B // NCORES  # batches per core
F32 = mybir.dt.float32
ADT = mybir.dt.bfloat16

SCJ = 8  # S/128 chunks (token/key chunks)
CCH = 4  # C/128 chunks (model-dim chunks)
FCH = 8  # (2C)/128 chunks of q|k features
VW = H * (D + 1)  # 520: v row width incl. ones column per head


def _register_ntff_hook():
    import sys, types

    if "antenv.axon_hooks" in sys.modules:
        return
    try:
        import trn_agent_boot.trn_boot as tb

        hook = [None]
        mod = types.ModuleType("antenv.axon_hooks")
        mod.set_axon_ntff_profile_hook = lambda h: hook.__setitem__(0, h)
        mod.get_axon_ntff_profile_hook = lambda: hook[0]
        sys.modules["antenv.axon_hooks"] = mod
        mod.set_axon_ntff_profile_hook(
            tb._ntff_profile_via_ctypes("/opt/axon/libaxon_pjrt.so")
        )
    except Exception:
        pass


def build():
    nc = bacc.Bacc("TRN2", target_bir_lowering=False, debug=False)

    xT = nc.declare_dram_parameter("xT", [BPC, C, S], ADT, isOutput=False)
    wqkvT = nc.declare_dram_parameter("wqkvT", [C, 3 * C], ADT, isOutput=False)
    wouT = nc.declare_dram_parameter("wouT", [C, C], ADT, isOutput=False)
    bqk = nc.declare_dram_parameter("bqk", [128, FCH], F32, isOutput=False)
    beff = nc.declare_dram_parameter("beff", [C], F32, isOutput=False)
    y = nc.declare_dram_parameter("y", [BPC, S, C], F32, isOutput=True)

    from contextlib import ExitStack

    with tile.TileContext(nc) as tc, ExitStack() as ctx:
        ctx.enter_context(
            nc.allow_low_precision(reason="bf16 matmul operand staging")
        )
        consts = ctx.enter_context(tc.tile_pool(name="consts", bufs=1))
        xpool = ctx.enter_context(tc.tile_pool(name="x", bufs=2))
        qkpool = ctx.enter_context(tc.tile_pool(name="qkt", bufs=17))
        vpool = ctx.enter_context(tc.tile_pool(name="v", bufs=2))
        ppool = ctx.enter_context(tc.tile_pool(name="p", bufs=4))
        opool = ctx.enter_context(tc.tile_pool(name="o", bufs=2))
        rpool = ctx.enter_context(tc.tile_pool(name="r", bufs=3))
        spool = ctx.enter_context(tc.tile_pool(name="s", bufs=2))
        ypool = ctx.enter_context(tc.tile_pool(name="y", bufs=11))
        bcpool = ctx.enter_context(tc.tile_pool(name="bc", bufs=3))
        drpool = ctx.enter_context(tc.tile_pool(name="dr", bufs=4, space="DRAM"))
        ps_a = ctx.enter_context(tc.tile_pool(name="ps_a", bufs=3, space="PSUM"))
        ps_o = ctx.enter_context(tc.tile_pool(name="ps_o", bufs=1, space="PSUM"))

        # --- first wave: x(b0) interleaved with q AND k thirds per cc so
        # the first scores' deps (fc0 + fc4) land earliest; v third next.
        wq_sb = consts.tile([128, CCH * 3 * C], ADT)  # [c%128, cc*1536 + f]
        bqk_sb = consts.tile([128, FCH], F32)
        nc.sync.dma_start(out=bqk_sb, in_=bqk[:, :])
        x0_sb = xpool.tile([128, CCH * S], ADT, tag="x", name="x0")
        for cc in range(CCH):
            nc.sync.dma_start(
                out=x0_sb[:, cc * S : (cc + 1) * S],
                in_=xT[0][cc * 128 : (cc + 1) * 128, :],
            )
            nc.gpsimd.dma_start(
                out=wq_sb[:, cc * 1536 : cc * 1536 + 512],
                in_=wqkvT[cc * 128 : (cc + 1) * 128, 0:512],
            )
            nc.gpsimd.dma_start(
                out=wq_sb[:, cc * 1536 + 512 : cc * 1536 + 1024],
                in_=wqkvT[cc * 128 : (cc + 1) * 128, 512:1024],
            )
        for cc in range(CCH):
            nc.gpsimd.dma_start(
                out=wq_sb[:, cc * 1536 + 1024 : cc * 1536 + 1536],
                in_=wqkvT[cc * 128 : (cc + 1) * 128, 1024:1536],
            )
        wo_sb = consts.tile([128, CCH * C], ADT)  # [c%128, cc*512 + f]
        nc.sync.dma_start(
            out=wo_sb.rearrange("p (cc f) -> p cc f", cc=CCH),
            in_=wouT[:, :].rearrange("(cc p) f -> p cc f", p=128),
        )
        beff_sb = consts.tile([128, C], F32)  # b_eff broadcast to all partitions
        nc.gpsimd.dma_start(out=beff_sb, in_=beff[:].partition_broadcast(128))
        # f32 1.0s: the transpose-matmul identity for the fast normalize
        # (rows 0 and 32 match the sums rows' base partitions)
        one_sb = consts.tile([33, 1], F32)
        nc.vector.memset(one_sb, 1.0)

        def emit_x(b):
            # x^T for batch b: [c, s] as [c%128, cc*1024 + s]
            x_sb = xpool.tile([128, CCH * S], ADT, tag="x", name=f"x{b}")
            for cc in range(CCH):
                nc.sync.dma_start(
                    out=x_sb[:, cc * S : (cc + 1) * S],
                    in_=xT[b][cc * 128 : (cc + 1) * 128, :],
                )
            return x_sb

        def emit_qk_chunk(b, x_sb, fc, qk):
            # q/k projection chunk: qkT[fc] = W_qk^T[:,fc].T @ x^T + b
            qt = qkpool.tile([128, S], ADT, tag="qkt", name=f"qkt{b}_{fc}")
            ps = ps_a.tile([128, 1024], F32, tag="ps_a", name=f"psq{b}_{fc}")
            for ih in range(2):
                for cc in range(CCH):
                    nc.tensor.matmul(
                        ps[:, ih * 512 : (ih + 1) * 512],
                        lhsT=wq_sb[:, cc * 1536 + fc * 128 : cc * 1536 + (fc + 1) * 128],
                        rhs=x_sb[:, cc * S + ih * 512 : cc * S + ih * 512 + 512],
                        start=(cc == 0),
                        stop=(cc == CCH - 1),
                    )
            # evacuate + bias (per-partition scalar add), cast to bf16
            nc.vector.tensor_scalar_add(
                out=qt, in0=ps[:, :], scalar1=bqk_sb[:, fc : fc + 1]
            )
            qk[fc] = qt

        def emit_v(b, x_sb):
            # v projection: token-major [s%128, jc*520 + h*65 + d], ones cols
            v_sb = vpool.tile([128, SCJ * VW], ADT, tag="v", name=f"v{b}")
            v_view = v_sb.rearrange("p (jc h dd) -> p jc h dd", jc=SCJ, h=H)
            nc.vector.memset(v_view[:, :, :, D : D + 1], 1.0)
            for jc in range(SCJ):
                ps = ps_a.tile([128, 1024], F32, tag="ps_a", name=f"psv{b}_{jc}")
                for cc in range(CCH):
                    nc.tensor.matmul(
                        ps[:, 0:512],
                        lhsT=x_sb[:, cc * S + jc * 128 : cc * S + (jc + 1) * 128],
                        rhs=wq_sb[:, cc * 1536 + 1024 : cc * 1536 + 1536],
                        start=(cc == 0),
                        stop=(cc == CCH - 1),
                    )
                nc.vector.tensor_copy(
                    out=v_view[:, jc, :, 0:D],
                    in_=ps[:, 0:512].rearrange("p (h d) -> p h d", h=H),
                )
            return v_sb

        pts = {}

        def emit_s(b, h, jc, qk_tiles):
            # scores chunk + exp; normally emitted inside emit_head, but the
            # first two chunks of head 0 are emitted early (right after the
            # fc0/fc4 projections) to start the ACT pipeline during the boot
            fq = h // 2
            fk = 4 + h // 2
            pb = (h % 2) * 64
            ps = ps_a.tile([128, 1024], F32, tag="ps_a", name=f"pss{b}_{h}_{jc}")
            for ih in range(2):
                nc.tensor.matmul(
                    ps[:, ih * 512 : (ih + 1) * 512],
                    lhsT=qk_tiles[fk][pb : pb + 64, jc * 128 : (jc + 1) * 128],
                    rhs=qk_tiles[fq][pb : pb + 64, ih * 512 : ih * 512 + 512],
                    start=True,
                    stop=True,
                )
            pt = ppool.tile([128, 1024], ADT, tag="p", name=f"pt{b}_{h}_{jc}")
            nc.scalar.activation(
                out=pt, in_=ps[:, :],
                func=mybir.ActivationFunctionType.Exp,
                scale=float(D) ** -0.5,
            )
            pts[(b, h, jc)] = pt

        def emit_head(b, h, qk_tiles, v_sb, o_sb, sums_sb):
            po = ps_o.tile([65, 1024], F32, tag="ps_o", name=f"po{b}_{h}")
            for jc in range(SCJ):
                if (b, h, jc) not in pts:
                    emit_s(b, h, jc, qk_tiles)
                pt = pts.pop((b, h, jc))
                # O^T[d, i] += V_ext^T @ P'  (row 64 = row-sums)
                for ih in range(2):
                    nc.tensor.matmul(
                        po[:, ih * 512 : (ih + 1) * 512],
                        lhsT=v_sb[:, jc * VW + h * (D + 1) : jc * VW + (h + 1) * (D + 1)],
                        rhs=pt[:, ih * 512 : (ih + 1) * 512],
                        start=(jc == 0),
                        stop=(jc == SCJ - 1),
                    )
            # evacuate row-sums first (they gate the normalize chain), then
            # the unnormalized O^T
            hh = h % 2
            nc.vector.tensor_copy(
                out=sums_sb[32 * hh : 32 * hh + 1, :], in_=po[64:65, :]
            )
            nc.vector.tensor_copy(
                out=o_sb[hh * 64 : (hh + 1) * 64, (h // 2) * S : (h // 2 + 1) * S],
                in_=po[0:64, :],
            )

        def emit_normalize_head(b, h, o_sb, sums_sb, fast=False):
            # normalize head h right after its po evac. Respread the sums
            # over 128 lanes (the DVE reciprocal is ~8 cyc/elem/lane):
            # normally via a DRAM bounce (cheap, ~13us latency, plenty of
            # slack); `fast` replaces the two front DMA hops with eight tiny
            # PE transpose-matmuls (stride-8 blocks so [p,c] = sums[p*8+c],
            # keeping the DMA out a 32-byte-run pattern) for the late heads
            # whose normalize gates the final output projection.
            hh = h % 2
            hp = h // 2
            if fast:
                sums_v = sums_sb.rearrange("p (q c) -> p q c", c=S // 128)
                ps1 = ps_a.tile([128, S // 128], F32, tag="ps_a", name=f"pst{b}_{h}")
                for c in range(S // 128):
                    nc.tensor.matmul(
                        ps1[:, c : c + 1],
                        lhsT=sums_v[32 * hh : 32 * hh + 1, :, c],
                        rhs=one_sb[32 * hh : 32 * hh + 1, 0:1],
                        is_transpose=True,
                        start=True,
                        stop=True,
                    )
                recs_sq = rpool.tile([128, S // 128], F32, tag="rsq", name=f"rsq{b}_{h}")
                nc.vector.reciprocal(out=recs_sq, in_=ps1)
            else:
                sums_dr = drpool.tile([S], F32, tag="sdr", name=f"sdr{b}_{h}")
                nc.sync.dma_start(
                    out=sums_dr[:].unsqueeze(0), in_=sums_sb[32 * hh : 32 * hh + 1, :]
                )
                sums_sq = rpool.tile([128, S // 128], F32, tag="ssq", name=f"ssq{b}_{h}")
                nc.sync.dma_start(
                    out=sums_sq, in_=sums_dr.rearrange("(p c) -> p c", p=128)
                )
                recs_sq = rpool.tile([128, S // 128], F32, tag="rsq", name=f"rsq{b}_{h}")
                nc.vector.reciprocal(out=recs_sq, in_=sums_sq)
            recs_dr = drpool.tile([S], F32, tag="rdr", name=f"rdr{b}_{h}")
            nc.sync.dma_start(
                out=recs_dr.rearrange("(p c) -> p c", p=128), in_=recs_sq
            )
            bc = bcpool.tile([128, S], F32, tag="bc", name=f"bc{b}_{h}")
            nc.sync.dma_start(
                out=bc[hh * 64 : (hh + 1) * 64, :],
                in_=recs_dr[:].partition_broadcast(64),
            )
            nc.vector.tensor_mul(
                out=o_sb[hh * 64 : (hh + 1) * 64, hp * S : (hp + 1) * S],
                in0=o_sb[hh * 64 : (hh + 1) * 64, hp * S : (hp + 1) * S],
                in1=bc[hh * 64 : (hh + 1) * 64, :],
            )

        def emit_outproj_chunk(b, o_sb, sc):
            ps = ps_a.tile([128, 1024], F32, tag="ps_a", name=f"psy{b}_{sc}")
            for cc in range(CCH):
                nc.tensor.matmul(
                    ps[:, 0:512],
                    lhsT=o_sb[:, cc * S + sc * 128 : cc * S + (sc + 1) * 128],
                    rhs=wo_sb[:, cc * C : (cc + 1) * C],
                    start=(cc == 0),
                    stop=(cc == CCH - 1),
                )
            y_sb = ypool.tile([128, C], F32, tag="y", name=f"y{b}_{sc}")
            nc.vector.tensor_add(out=y_sb, in0=ps[:, 0:512], in1=beff_sb)
            # gpsimd queue: keep sync free for the normalize chain hops
            nc.gpsimd.dma_start(out=y[b][sc * 128 : (sc + 1) * 128, :], in_=y_sb)

        ys = [None] * SCJ

        def emit_pyA(o_sb, sc):
            # final batch out-proj, phase A: cc=0..2 staged into SBUF
            ps = ps_a.tile([128, 512], F32, tag="ps_a", name=f"pyA{sc}")
            for cc in range(CCH - 1):
                nc.tensor.matmul(
                    ps[:, 0:512],
                    lhsT=o_sb[:, cc * S + sc * 128 : cc * S + (sc + 1) * 128],
                    rhs=wo_sb[:, cc * C : (cc + 1) * C],
                    start=(cc == 0),
                    stop=(cc == CCH - 2),
                )
            y_sb = ypool.tile([128, C], F32, tag="y", name=f"yA{sc}")
            nc.vector.tensor_add(out=y_sb, in0=ps[:, 0:512], in1=beff_sb)
            ys[sc] = y_sb

        def emit_pyB(o_sb, sc):
            # final batch out-proj, phase B: cc=3 added into the staged
            # tiles; stores split across both queues to halve the drain
            cc = CCH - 1
            ps = ps_a.tile([128, 512], F32, tag="ps_a", name=f"pyB{sc}")
            nc.tensor.matmul(
                ps[:, 0:512],
                lhsT=o_sb[:, cc * S + sc * 128 : cc * S + (sc + 1) * 128],
                rhs=wo_sb[:, cc * C : (cc + 1) * C],
                start=True,
                stop=True,
            )
            nc.vector.tensor_add(out=ys[sc], in0=ys[sc], in1=ps[:, 0:512])
            nc.gpsimd.dma_start(
                out=y[BPC - 1][sc * 128 : (sc + 1) * 128, 0:256], in_=ys[sc][:, 0:256]
            )
            nc.sync.dma_start(
                out=y[BPC - 1][sc * 128 : (sc + 1) * 128, 256:512],
                in_=ys[sc][:, 256:512],
            )

        def emit_proj(b, x_sb):
            qk = [None] * FCH
            emit_qk_chunk(b, x_sb, 0, qk)
            emit_qk_chunk(b, x_sb, 4, qk)
            if b == 0:
                # head 0's first scores + exps start during the remaining
                # projections, pulling the ACT pipeline start earlier
                emit_s(0, 0, 0, qk)
                emit_s(0, 0, 1, qk)
            for fc in (1, 5, 2, 6, 3, 7):
                emit_qk_chunk(b, x_sb, fc, qk)
            v_sb = emit_v(b, x_sb)
            return x_sb, qk, v_sb

        state = emit_proj(0, x0_sb)
        o_sbs = []
        nxt_x, nxt_qk = None, [None] * FCH
        for b in range(BPC):
            x_sb, qk_tiles, v_sb = state
            o_sb = opool.tile([128, CCH * S], ADT, tag="o", name=f"o{b}")
            o_sbs.append(o_sb)
            last_b = b == BPC - 1
            for hp in range(H // 2):
                sums_sb = spool.tile([33, 1024], F32, tag="sums", name=f"sm{b}_{hp}")
                emit_head(b, 2 * hp, qk_tiles, v_sb, o_sb, sums_sb)
                if last_b and hp == H // 2 - 1:
                    # tail: head 6's bounce-normalize runs while phase A of
                    # the out-projection (needs heads 0..5 only) fills the
                    # PE; then head 7, the last two batch-0 chunks, head 7's
                    # fast normalize, and the cc=3 phase B.
                    emit_normalize_head(b, 6, o_sb, sums_sb)
                    for sc in range(SCJ):
                        emit_pyA(o_sb, sc)
                    emit_head(b, 7, qk_tiles, v_sb, o_sb, sums_sb)
                    emit_outproj_chunk(b - 1, o_sbs[b - 1], 6)
                    emit_outproj_chunk(b - 1, o_sbs[b - 1], 7)
                    emit_normalize_head(b, 7, o_sb, sums_sb, fast=True)
                    for sc in range(SCJ):
                        emit_pyB(o_sb, sc)
                    break
                emit_normalize_head(b, 2 * hp, o_sb, sums_sb)
                emit_head(b, 2 * hp + 1, qk_tiles, v_sb, o_sb, sums_sb)
                emit_normalize_head(
                    b, 2 * hp + 1, o_sb, sums_sb, fast=(last_b and hp == 2)
                )
                if not last_b:
                    if hp == 0:
                        nxt_x = emit_x(b + 1)
                    emit_qk_chunk(b + 1, nxt_x, (0, 4, 1, 5, 2, 6, 3, 7)[2 * hp], nxt_qk)
                    emit_qk_chunk(b + 1, nxt_x, (0, 4, 1, 5, 2, 6, 3, 7)[2 * hp + 1], nxt_qk)
                    if hp == H // 2 - 1:
                        state = (nxt_x, nxt_qk, emit_v(b + 1, nxt_x))
                else:
                    # fill the projection slots with batch b-1's out-proj
                    emit_outproj_chunk(b - 1, o_sbs[b - 1], 2 * hp)
                    emit_outproj_chunk(b - 1, o_sbs[b - 1], 2 * hp + 1)

    nc.compile()
    return nc


_NC_CACHE = None
LAST_RESULT = None


def kernel(vis_feat, text_feat, w_qkv, b_qkv, w_out, b_out):
    global _NC_CACHE, LAST_RESULT
    _register_ntff_hook()
    if _NC_CACHE is None:
        _NC_CACHE = build()
    nc = _NC_CACHE

    adt_np = np.dtype(mybir.dt.np(ADT))
    vis_feat = np.asarray(vis_feat, dtype=np.float32)
    w_qkv = np.asarray(w_qkv, dtype=np.float32)
    b_qkv = np.asarray(b_qkv, dtype=np.float32)
    w_out = np.asarray(w_out, dtype=np.float32)
    b_out = np.asarray(b_out, dtype=np.float32)

    wqkvT = np.ascontiguousarray(w_qkv.T).astype(adt_np)  # [C, 3C]
    wouT = np.ascontiguousarray(w_out.T).astype(adt_np)  # [C, C]
    bqk = np.ascontiguousarray(b_qkv[: 2 * C].reshape(FCH, 128).T)  # [128, 8]
    beff = np.ascontiguousarray(b_out + b_qkv[2 * C :] @ w_out.T)  # [C]

    in_maps = []
    for i in range(NCORES):
        xTi = np.ascontiguousarray(
            vis_feat[i * BPC : (i + 1) * BPC].transpose(0, 2, 1)
        ).astype(adt_np)  # [BPC, C, S]
        in_maps.append(
            {"xT": xTi, "wqkvT": wqkvT, "wouT": wouT, "bqk": bqk, "beff": beff}
        )

    res = run_bass_kernel_spmd(nc, in_maps, core_ids=list(range(NCORES)))
    LAST_RESULT = res
    return np.concatenate([res.results[i]["y"] for i in range(NCORES)], axis=0)


# revision 30
# speedup vs baseline: 1.2334x; 1.0590x over previous
"""Self-attention block (B=16, S=1024, C=512, H=8, D=64) on 8 NeuronCores.

Data-parallel over batch: core i handles batches [2i, 2i+1]. No collectives.

Per-core pipeline (all on-chip after the initial DMAs):
  qkv proj -> q,k feature-major [d, s], v token-major [s, d] with a ones
  column per head (so P@V_ext also yields the softmax row-sums); scores
  computed transposed S'[j, i] = k . q so exp(S') feeds the P@V matmul
  directly as lhsT; softmax skips max-subtraction (logits bounded ~+-4);
  deferred per-head normalization divides O^T rows by the row-sums
  (respread over 128 lanes via a DRAM bounce for the DVE reciprocal,
  broadcast back, in-place multiply); output projection consumes the
  normalized heads straight out of SBUF. The value-path bias is folded
  through attention into the output bias (exact: softmax rows sum to 1).

Scheduling: initial DMAs interleave x chunks with q AND k thirds of
w_qkv per-cc across two queues (k was previously last and gated the
first scores by ~3us); the first two scores chunks of head 0 are emitted
right after the fc0/fc4 projections so the exp pipeline starts during
the remaining projections; batch 1's projection chunks fill the PE slack
between batch 0's attention pairs, batch 0's output projection fills the
same slots during batch 1; heads normalize right after their P@V
evacuation so the chain hides under the next head. Final-batch tail:
head 6's normalize bounces through DRAM while phase A of the output
projection (cc=0..2, staged into SBUF) runs between heads 6 and 7;
heads 5 and 7 use a lower-latency normalize whose lane-respread is eight
tiny PE transpose-matmuls instead of the two front DMA hops; after head
7, the last two batch-0 projection chunks cover the normalize chain, and
the cc=3 contributions land into the staged tiles with stores split
across both DMA queues.

Dtypes: bf16 matmuls throughout. fp8 (DoubleRow) was evaluated end to
end: softmax averaging shrinks signal as fast as quantization noise, so
the ~4% fp8e4m3 element error survives to the output (rel-err ~2e-2 vs
the 2e-2 gate) -- fundamentally unusable here. NOTE: the chip enforces a
package power cap with DVFS: schedules that pack the PE past ~80% busy
(removing the per-head pipeline-refill gaps, adding warm-up matmuls,
etc.) LOWER the average PE clock and run strictly slower -- the
per-head gaps are load-bearing; leave them.
"""

import numpy as np

import concourse.bacc as bacc
import concourse.tile as tile
import concourse.mybir as mybir
from concourse.bass_utils import run_bass_kernel_spmd

B, S, C, H, D = 16, 1024, 512, 8, 64
NCORES = 8
BPC = B // NCORES  # batches per core
F32 = mybir.dt.float32
ADT = mybir.dt.bfloat16

SCJ = 8  # S/128 chunks (token/key chunks)
CCH = 4  # C/128 chunks (model-dim chunks)
FCH = 8  # (2C)/128 chunks of q|k features
VW = H * (D + 1)  # 520: v row width incl. ones column per head


def _register_ntff_hook():
    import sys, types

    if "antenv.axon_hooks" in sys.modules:
        return
    try:
        import trn_agent_boot.trn_boot as tb

        hook = [None]
        mod = types.ModuleType("antenv.axon_hooks")
        mod.set_axon_ntff_profile_hook = lambda h: hook.__setitem__(0, h)
        mod.get_axon_ntff_profile_hook = lambda: hook[0]
        sys.modules["antenv.axon_hooks"] = mod
        mod.set_axon_ntff_profile_hook(
            tb._ntff_profile_via_ctypes("/opt/axon/libaxon_pjrt.so")
        )
    except Exception:
        pass


def build():
    nc = bacc.Bacc("TRN2", target_bir_lowering=False, debug=False)

    xT = nc.declare_dram_parameter("xT", [BPC, C, S], ADT, isOutput=False)
    wqkvT = nc.declare_dram_parameter("wqkvT", [C, 3 * C], ADT, isOutput=False)
    wouT = nc.declare_dram_parameter("wouT", [C, C], ADT, isOutput=False)
    bqk = nc.declare_dram_parameter("bqk", [128, FCH], F32, isOutput=False)
    beff = nc.declare_dram_parameter("beff", [C], F32, isOutput=False)
    y = nc.declare_dram_parameter("y", [BPC, S, C], F32, isOutput=True)

    from contextlib import ExitStack

    with tile.TileContext(nc) as tc, ExitStack() as ctx:
        ctx.enter_context(
            nc.allow_low_precision(reason="bf16 matmul operand staging")
        )
        consts = ctx.enter_context(tc.tile_pool(name="consts", bufs=1))
        xpool = ctx.enter_context(tc.tile_pool(name="x", bufs=2))
        qkpool = ctx.enter_context(tc.tile_pool(name="qkt", bufs=17))
        vpool = ctx.enter_context(tc.tile_pool(name="v", bufs=2))
        ppool = ctx.enter_context(tc.tile_pool(name="p", bufs=4))
        opool = ctx.enter_context(tc.tile_pool(name="o", bufs=2))
        rpool = ctx.enter_context(tc.tile_pool(name="r", bufs=3))
        spool = ctx.enter_context(tc.tile_pool(name="s", bufs=2))
        ypool = ctx.enter_context(tc.tile_pool(name="y", bufs=11))
        bcpool = ctx.enter_context(tc.tile_pool(name="bc", bufs=3))
        drpool = ctx.enter_context(tc.tile_pool(name="dr", bufs=4, space="DRAM"))
        ps_a = ctx.enter_context(tc.tile_pool(name="ps_a", bufs=3, space="PSUM"))
        ps_o = ctx.enter_context(tc.tile_pool(name="ps_o", bufs=1, space="PSUM"))

        # --- first wave: x(b0) interleaved with q AND k thirds per cc so
        # the first scores' deps (fc0 + fc4) land earliest; v third next.
        wq_sb = consts.tile([128, CCH * 3 * C], ADT)  # [c%128, cc*1536 + f]
        bqk_sb = consts.tile([128, FCH], F32)
        nc.sync.dma_start(out=bqk_sb, in_=bqk[:, :])
        x0_sb = xpool.tile([128, CCH * S], ADT, tag="x", name="x0")
        for cc in range(CCH):
            nc.sync.dma_start(
                out=x0_sb[:, cc * S : (cc + 1) * S],
                in_=xT[0][cc * 128 : (cc + 1) * 128, :],
            )
            nc.gpsimd.dma_start(
                out=wq_sb[:, cc * 1536 : cc * 1536 + 512],
                in_=wqkvT[cc * 128 : (cc + 1) * 128, 0:512],
            )
            nc.gpsimd.dma_start(
                out=wq_sb[:, cc * 1536 + 512 : cc * 1536 + 1024],
                in_=wqkvT[cc * 128 : (cc + 1) * 128, 512:1024],
            )
        for cc in range(CCH):
            nc.gpsimd.dma_start(
                out=wq_sb[:, cc * 1536 + 1024 : cc * 1536 + 1536],
                in_=wqkvT[cc * 128 : (cc + 1) * 128, 1024:1536],
            )
        wo_sb = consts.tile([128, CCH * C], ADT)  # [c%128, cc*512 + f]
        nc.sync.dma_start(
            out=wo_sb.rearrange("p (cc f) -> p cc f", cc=CCH),
            in_=wouT[:, :].rearrange("(cc p) f -> p cc f", p=128),
        )
        beff_sb = consts.tile([128, C], F32)  # b_eff broadcast to all partitions
        nc.gpsimd.dma_start(out=beff_sb, in_=beff[:].partition_broadcast(128))
        # f32 1.0s: the transpose-matmul identity for the fast normalize
        # (rows 0 and 32 match the sums rows' base partitions)
        one_sb = consts.tile([33, 1], F32)
        nc.vector.memset(one_sb, 1.0)

        def emit_x(b):
            # x^T for batch b: [c, s] as [c%128, cc*1024 + s]
            x_sb = xpool.tile([128, CCH * S], ADT, tag="x", name=f"x{b}")
            for cc in range(CCH):
                nc.sync.dma_start(
                    out=x_sb[:, cc * S : (cc + 1) * S],
                    in_=xT[b][cc * 128 : (cc + 1) * 128, :],
                )
            return x_sb

        def emit_qk_chunk(b, x_sb, fc, qk):
            # q/k projection chunk: qkT[fc] = W_qk^T[:,fc].T @ x^T + b
            qt = qkpool.tile([128, S], ADT, tag="qkt", name=f"qkt{b}_{fc}")
            ps = ps_a.tile([128, 1024], F32, tag="ps_a", name=f"psq{b}_{fc}")
            for ih in range(2):
                for cc in range(CCH):
                    nc.tensor.matmul(
                        ps[:, ih * 512 : (ih + 1) * 512],
                        lhsT=wq_sb[:, cc * 1536 + fc * 128 : cc * 1536 + (fc + 1) * 128],
                        rhs=x_sb[:, cc * S + ih * 512 : cc * S + ih * 512 + 512],
                        start=(cc == 0),
                        stop=(cc == CCH - 1),
                    )
            # evacuate + bias (per-partition scalar add), cast to bf16
            nc.vector.tensor_scalar_add(
                out=qt, in0=ps[:, :], scalar1=bqk_sb[:, fc : fc + 1]
            )
            qk[fc] = qt

        def emit_v(b, x_sb):
            # v projection: token-major [s%128, jc*520 + h*65 + d], ones cols
            v_sb = vpool.tile([128, SCJ * VW], ADT, tag="v", name=f"v{b}")
            v_view = v_sb.rearrange("p (jc h dd) -> p jc h dd", jc=SCJ, h=H)
            nc.vector.memset(v_view[:, :, :, D : D + 1], 1.0)
            for jc in range(SCJ):
                ps = ps_a.tile([128, 1024], F32, tag="ps_a", name=f"psv{b}_{jc}")
                for cc in range(CCH):
                    nc.tensor.matmul(
                        ps[:, 0:512],
                        lhsT=x_sb[:, cc * S + jc * 128 : cc * S + (jc + 1) * 128],
                        rhs=wq_sb[:, cc * 1536 + 1024 : cc * 1536 + 1536],
                        start=(cc == 0),
                        stop=(cc == CCH - 1),
                    )
                nc.vector.tensor_copy(
                    out=v_view[:, jc, :, 0:D],
                    in_=ps[:, 0:512].rearrange("p (h d) -> p h d", h=H),
                )
            return v_sb

        pts = {}

        def emit_s(b, h, jc, qk_tiles):
            # scores chunk + exp; normally emitted inside emit_head, but the
            # first two chunks of head 0 are emitted early (right after the
            # fc0/fc4 projections) to start the ACT pipeline during the boot
            fq = h // 2
            fk = 4 + h // 2
            pb = (h % 2) * 64
            ps = ps_a.tile([128, 1024], F32, tag="ps_a", name=f"pss{b}_{h}_{jc}")
            for ih in range(2):
                nc.tensor.matmul(
                    ps[:, ih * 512 : (ih + 1) * 512],
                    lhsT=qk_tiles[fk][pb : pb + 64, jc * 128 : (jc + 1) * 128],
                    rhs=qk_tiles[fq][pb : pb + 64, ih * 512 : ih * 512 + 512],
                    start=True,
                    stop=True,
                )
            pt = ppool.tile([128, 1024], ADT, tag="p", name=f"pt{b}_{h}_{jc}")
            nc.scalar.activation(
                out=pt, in_=ps[:, :],
                func=mybir.ActivationFunctionType.Exp,
                scale=float(D) ** -0.5,
            )
            pts[(b, h, jc)] = pt

        def emit_head(b, h, qk_tiles, v_sb, o_sb, sums_sb):
            po = ps_o.tile([65, 1024], F32, tag="ps_o", name=f"po{b}_{h}")
            for jc in range(SCJ):
                if (b, h, jc) not in pts:
                    emit_s(b, h, jc, qk_tiles)
                pt = pts.pop((b, h, jc))
                # O^T[d, i] += V_ext^T @ P'  (row 64 = row-sums)
                for ih in range(2):
                    nc.tensor.matmul(
                        po[:, ih * 512 : (ih + 1) * 512],
                        lhsT=v_sb[:, jc * VW + h * (D + 1) : jc * VW + (h + 1) * (D + 1)],
                        rhs=pt[:, ih * 512 : (ih + 1) * 512],
                        start=(jc == 0),
                        stop=(jc == SCJ - 1),
                    )
            # evacuate row-sums first (they gate the normalize chain), then
            # the unnormalized O^T
            hh = h % 2
            nc.vector.tensor_copy(
                out=sums_sb[32 * hh : 32 * hh + 1, :], in_=po[64:65, :]
            )
            nc.vector.tensor_copy(
                out=o_sb[hh * 64 : (hh + 1) * 64, (h // 2) * S : (h // 2 + 1) * S],
                in_=po[0:64, :],
            )

        def emit_normalize_head(b, h, o_sb, sums_sb, fast=False):
            # normalize head h right after its po evac. Respread the sums
            # over 128 lanes (the DVE reciprocal is ~8 cyc/elem/lane):
            # normally via a DRAM bounce (cheap, ~13us latency, plenty of
            # slack); `fast` replaces the two front DMA hops with eight tiny
            # PE transpose-matmuls (stride-8 blocks so [p,c] = sums[p*8+c],
            # keeping the DMA out a 32-byte-run pattern) for the late heads
            # whose normalize gates the final output projection.
            hh = h % 2
            hp = h // 2
            if fast:
                sums_v = sums_sb.rearrange("p (q c) -> p q c", c=S // 128)
                ps1 = ps_a.tile([128, S // 128], F32, tag="ps_a", name=f"pst{b}_{h}")
                for c in range(S // 128):
                    nc.tensor.matmul(
                        ps1[:, c : c + 1],
                        lhsT=sums_v[32 * hh : 32 * hh + 1, :, c],
                        rhs=one_sb[32 * hh : 32 * hh + 1, 0:1],
                        is_transpose=True,
                        start=True,
                        stop=True,
                    )
                recs_sq = rpool.tile([128, S // 128], F32, tag="rsq", name=f"rsq{b}_{h}")
                nc.vector.reciprocal(out=recs_sq, in_=ps1)
            else:
                sums_dr = drpool.tile([S], F32, tag="sdr", name=f"sdr{b}_{h}")
                nc.sync.dma_start(
                    out=sums_dr[:].unsqueeze(0), in_=sums_sb[32 * hh : 32 * hh + 1, :]
                )
                sums_sq = rpool.tile([128, S // 128], F32, tag="ssq", name=f"ssq{b}_{h}")
                nc.sync.dma_start(
                    out=sums_sq, in_=sums_dr.rearrange("(p c) -> p c", p=128)
                )
                recs_sq = rpool.tile([128, S // 128], F32, tag="rsq", name=f"rsq{b}_{h}")
                nc.vector.reciprocal(out=recs_sq, in_=sums_sq)
            recs_dr = drpool.tile([S], F32, tag="rdr", name=f"rdr{b}_{h}")
            nc.sync.dma_start(
                out=recs_dr.rearrange("(p c) -> p c", p=128), in_=recs_sq
            )
            bc = bcpool.tile([128, S], F32, tag="bc", name=f"bc{b}_{h}")
            nc.sync.dma_start(
                out=bc[hh * 64 : (hh + 1) * 64, :],
                in_=recs_dr[:].partition_broadcast(64),
            )
            nc.vector.tensor_mul(
                out=o_sb[hh * 64 : (hh + 1) * 64, hp * S : (hp + 1) * S],
                in0=o_sb[hh * 64 : (hh + 1) * 64, hp * S : (hp + 1) * S],
                in1=bc[hh * 64 : (hh + 1) * 64, :],
            )

        def emit_outproj_chunk(b, o_sb, sc):
            ps = ps_a.tile([128, 1024], F32, tag="ps_a", name=f"psy{b}_{sc}")
            for cc in range(CCH):
                nc.tensor.matmul(
                    ps[:, 0:512],
                    lhsT=o_sb[:, cc * S + sc * 128 : cc * S + (sc + 1) * 128],
                    rhs=wo_sb[:, cc * C : (cc + 1) * C],
                    start=(cc == 0),
                    stop=(cc == CCH - 1),
                )
            y_sb = ypool.tile([128, C], F32, tag="y", name=f"y{b}_{sc}")
            nc.vector.tensor_add(out=y_sb, in0=ps[:, 0:512], in1=beff_sb)
            # gpsimd queue: keep sync free for the normalize chain hops
            nc.gpsimd.dma_start(out=y[b][sc * 128 : (sc + 1) * 128, :], in_=y_sb)

        ys = [None] * SCJ

        def emit_pyA(o_sb, sc):
            # final batch out-proj, phase A: cc=0..2 staged into SBUF
            ps = ps_a.tile([128, 512], F32, tag="ps_a", name=f"pyA{sc}")
            for cc in range(CCH - 1):
                nc.tensor.matmul(
                    ps[:, 0:512],
                    lhsT=o_sb[:, cc * S + sc * 128 : cc * S + (sc + 1) * 128],
                    rhs=wo_sb[:, cc * C : (cc + 1) * C],
                    start=(cc == 0),
                    stop=(cc == CCH - 2),
                )
            y_sb = ypool.tile([128, C], F32, tag="y", name=f"yA{sc}")
            nc.vector.tensor_add(out=y_sb, in0=ps[:, 0:512], in1=beff_sb)
            ys[sc] = y_sb

        def emit_pyB(o_sb, sc):
            # final batch out-proj, phase B: cc=3 added into the staged
            # tiles; stores split across both queues to halve the drain
            cc = CCH - 1
            ps = ps_a.tile([128, 512], F32, tag="ps_a", name=f"pyB{sc}")
            nc.tensor.matmul(
                ps[:, 0:512],
                lhsT=o_sb[:, cc * S + sc * 128 : cc * S + (sc + 1) * 128],
                rhs=wo_sb[:, cc * C : (cc + 1) * C],
                start=True,
                stop=True,
            )
            nc.vector.tensor_add(out=ys[sc], in0=ys[sc], in1=ps[:, 0:512])
            nc.gpsimd.dma_start(
                out=y[BPC - 1][sc * 128 : (sc + 1) * 128, 0:256], in_=ys[sc][:, 0:256]
            )
            nc.sync.dma_start(
                out=y[BPC - 1][sc * 128 : (sc + 1) * 128, 256:512],
                in_=ys[sc][:, 256:512],
            )

        def emit_proj(b, x_sb):
            qk = [None] * FCH
            emit_qk_chunk(b, x_sb, 0, qk)
            emit_qk_chunk(b, x_sb, 4, qk)
            if b == 0:
                # head 0's first scores + exps start during the remaining
                # projections, pulling the ACT pipeline start earlier
                emit_s(0, 0, 0, qk)
                emit_s(0, 0, 1, qk)
            for fc in (1, 5, 2, 6, 3, 7):
                emit_qk_chunk(b, x_sb, fc, qk)
            v_sb = emit_v(b, x_sb)
            return x_sb, qk, v_sb

        state = emit_proj(0, x0_sb)
        o_sbs = []
        nxt_x, nxt_qk = None, [None] * FCH
        for b in range(BPC):
            x_sb, qk_tiles, v_sb = state
            o_sb = opool.tile([128, CCH * S], ADT, tag="o", name=f"o{b}")
            o_sbs.append(o_sb)
            last_b = b == BPC - 1
            for hp in range(H // 2):
                sums_sb = spool.tile([33, 1024], F32, tag="sums", name=f"sm{b}_{hp}")
                emit_head(b, 2 * hp, qk_tiles, v_sb, o_sb, sums_sb)
                if last_b and hp == H // 2 - 1:
                    # tail: after head 7, the last two batch-0 projection
                    # chunks cover head 7's sums evacuation, then its fast
                    # normalize (PE-transpose respread, ~9us instead of the
                    # ~14us bounce) overlaps phase A of the out-projection;
                    # phase B (cc=3) lands into the staged tiles.
                    emit_normalize_head(b, 6, o_sb, sums_sb)
                    emit_head(b, 7, qk_tiles, v_sb, o_sb, sums_sb)
                    emit_outproj_chunk(b - 1, o_sbs[b - 1], 6)
                    emit_outproj_chunk(b - 1, o_sbs[b - 1], 7)
                    emit_normalize_head(b, 7, o_sb, sums_sb, fast=True)
                    for sc in range(SCJ):
                        emit_pyA(o_sb, sc)
                    for sc in range(SCJ):
                        emit_pyB(o_sb, sc)
                    break
                emit_normalize_head(b, 2 * hp, o_sb, sums_sb)
                emit_head(b, 2 * hp + 1, qk_tiles, v_sb, o_sb, sums_sb)
                emit_normalize_head(b, 2 * hp + 1, o_sb, sums_sb)
                if not last_b:
                    if hp == 0:
                        nxt_x = emit_x(b + 1)
                    emit_qk_chunk(b + 1, nxt_x, (0, 4, 1, 5, 2, 6, 3, 7)[2 * hp], nxt_qk)
                    emit_qk_chunk(b + 1, nxt_x, (0, 4, 1, 5, 2, 6, 3, 7)[2 * hp + 1], nxt_qk)
                    if hp == H // 2 - 1:
                        state = (nxt_x, nxt_qk, emit_v(b + 1, nxt_x))
                else:
                    # fill the projection slots with batch b-1's out-proj
                    emit_outproj_chunk(b - 1, o_sbs[b - 1], 2 * hp)
                    emit_outproj_chunk(b - 1, o_sbs[b - 1], 2 * hp + 1)

    nc.compile()
    return nc


_NC_CACHE = None
LAST_RESULT = None


def kernel(vis_feat, text_feat, w_qkv, b_qkv, w_out, b_out):
    global _NC_CACHE, LAST_RESULT
    _register_ntff_hook()
    if _NC_CACHE is None:
        _NC_CACHE = build()
    nc = _NC_CACHE

    adt_np = np.dtype(mybir.dt.np(ADT))
    vis_feat = np.asarray(vis_feat, dtype=np.float32)
    w_qkv = np.asarray(w_qkv, dtype=np.float32)
    b_qkv = np.asarray(b_qkv, dtype=np.float32)
    w_out = np.asarray(w_out, dtype=np.float32)
    b_out = np.asarray(b_out, dtype=np.float32)

    wqkvT = np.ascontiguousarray(w_qkv.T).astype(adt_np)  # [C, 3C]
    wouT = np.ascontiguousarray(w_out.T).astype(adt_np)  # [C, C]
    bqk = np.ascontiguousarray(b_qkv[: 2 * C].reshape(FCH, 128).T)  # [128, 8]
    beff = np.ascontiguousarray(b_out + b_qkv[2 * C :] @ w_out.T)  # [C]

    in_maps = []
    for i in range(NCORES):
        xTi = np.ascontiguousarray(
            vis_feat[i * BPC : (i + 1) * BPC].transpose(0, 2, 1)
        ).astype(adt_np)  # [BPC, C, S]
        in_maps.append(
            {"xT": xTi, "wqkvT": wqkvT, "wouT": wouT, "bqk": bqk, "beff": beff}
        )

    res = run_bass_kernel_spmd(nc, in_maps, core_ids=list(range(NCORES)))
    LAST_RESULT = res
    return np.concatenate([res.results[i]["y"] for i in range(NCORES)], axis=0)
